# revision 31
# baseline (speedup 1.0000x reference)
"""CEDiceMetrics Trainium2 kernel (nn_CEDiceMetrics_69148973466078).

Computes dice/tp/psum/tsum for input [2,8,128,192,192] f32 logits and
target [2,1,128,192,192] int32 labels, sharded over 8 NeuronCores by
splitting the flattened voxel dim.

Per-core device algorithm (tiles of [128 partitions, FD] voxels):
  1. A custom fused DVE op (EMBED_MAX_ANT, declared below and registered
     into the ant custom-DVE table) embeds each channel id into the 3 low
     mantissa bits of the f32 logits and takes the pairwise max in a
     single pass: out = max(a ^ ((a&7) ^ idA), b ^ ((b&7) ^ idB))
     (= (x & ~7) | id on raw bits; XOR form avoids a NaN-pattern
     constant, which HW canonicalizes). All embedded values are distinct
     so the max is tie-free; the <=8-ulp perturbation can only flip
     argmax winners for near-exact ties (~1e-4 relative count error worst
     case, far inside tolerance).
  2. Three more plain f32 max ops fold the four pair-maxes -> m;
     pred = (m as int32) & 7 is the argmax index, exactly.
  3. comb = 8*tgt + pred, kept in bf16 (values 0..63 exact) so the
     per-class tp masks [comb == 9c] run in the DVE 4x perf mode.
  4. tp[c]: bf16 masks are column-summed by the otherwise-idle TensorE
     (ones-vector matmuls accumulating into PSUM; a one-hot lhsT column
     routes each batch to its own PSUM row).
  5. psum/tsum via cumulative counts on the Scalar engine: Sign(v-(c+.5))
     with fused accum_out gives S = N - 2*count(v <= c); class counts are
     differences of adjacent cumulative counts (host side). Chunked so
     the tail threshold work overlaps compute.
Host glue sums the tiny per-core/per-partition partial counts and
evaluates the dice formula. Measured ~156 us on HW vs ~118 us HBM
roofline (42.5 MB/core @ ~358 GB/s); VectorE-bound.
"""

import sys

for _p in ("/root/.axon_site/_ro/trn_rl_repo",):
    if _p not in sys.path:
        sys.path.insert(0, _p)

import numpy as np
from contextlib import ExitStack

import concourse.bacc as bacc
import concourse.mybir as mybir
import concourse.tile as tile
from concourse.bass_utils import run_bass_kernel_spmd
import concourse.dve_ops as _dve_ops
from concourse.dve_ops import DveOp as _DveOp
from concourse.dve_spec import (AluOp as _AluOp, Bin as _Bin, Spec as _Spec,
                                Src0 as _Src0, Src1 as _Src1, C0 as _C0,
                                C1 as _C1, C2 as _C2)


def _embed_max_ref(in0, in1, s0, s1, imm2):
    # z = x ^ ((x & 7) ^ id)  ==  (x & ~7) | id on raw f32 bits
    sev = np.asarray(s0, np.float32).view(np.int32)
    ia = np.asarray(s1, np.float32).view(np.int32)
    ib = np.float32(imm2).view(np.int32)
    xa = np.asarray(in0, np.float32).view(np.int32)
    xb = np.asarray(in1, np.float32).view(np.int32)
    a = (xa ^ ((xa & sev) ^ ia)).view(np.float32)
    b = (xb ^ ((xb & sev) ^ ib)).view(np.float32)
    return np.maximum(a, b)


def _make_embed_max():
    import re as _re
    name = "EMBED_MAX_ANT"
    body = _Bin(
        _AluOp.MAX,
        _Bin(_AluOp.BITWISE_XOR, _Src0,
             _Bin(_AluOp.BITWISE_XOR, _Bin(_AluOp.BITWISE_AND, _Src0, _C0),
                  _C1)),
        _Bin(_AluOp.BITWISE_XOR, _Src1,
             _Bin(_AluOp.BITWISE_XOR, _Bin(_AluOp.BITWISE_AND, _Src1, _C0),
                  _C2)),
    )
    spec = _Spec(body=body, reference=_embed_max_ref)
    for op in _dve_ops.OPS:
        if op.name == name:
            return op
    if name not in _dve_ops._SUB_OPCODE_FOR_NAME:
        _dve_ops._SUB_OPCODE_FOR_NAME[name] = (
            max(_dve_ops._SUB_OPCODE_FOR_NAME.values()) + 1)
    probe = _DveOp(name, spec, subdim=False, uops_sha={})
    shas = {}
    try:
        probe.compile("v3")
    except ValueError as e:
        shas["v3"] = _re.search(r"v3: (\w+)", str(e)).group(1)
    op = _DveOp(name, spec, subdim=False, uops_sha=shas)
    _dve_ops.OPS.append(op)
    _dve_ops.CUSTOM_DVE_SPECS[name] = spec
    return op


_EMBED_MAX = _make_embed_max()


def _id_bits_float(i):
    return float(np.int32(i).view(np.float32))

# Problem geometry (hardcoded per spec).
B, C = 2, 8
D, H, W = 128, 192, 192
N = D * H * W                 # 4,718,592 voxels per (b, c)
NCORES = 8
NV = N // NCORES              # 589,824 voxels per core per batch
P = 128
FDC = NV // P                 # 4,608 free elems per partition per batch
FD = 1536                     # free elems per round
RPB = FDC // FD               # 3 rounds per batch
R = B * RPB                   # 6 rounds per core
EPS = 1e-5

_CACHE = {}


MM_N = 512                      # PSUM bank width in f32; matmul chunk
MM_CHUNKS = FDC // MM_N         # matmul chunks per batch-level mask


def _build_nc(with_bin0=False):
    nc = bacc.Bacc("TRN2", target_bir_lowering=False, debug=False,
                   num_devices=NCORES)
    x_dram = nc.dram_tensor("x", [B * C * P, FDC], mybir.dt.float32,
                            kind="ExternalInput")
    t_dram = nc.dram_tensor("tgt", [B * P, FDC], mybir.dt.int32,
                            kind="ExternalInput")
    tp_dram = nc.dram_tensor("tp_o", [B, C], mybir.dt.float32,
                             kind="ExternalOutput")
    ps_dram = nc.dram_tensor("ps_o", [P, 2 * B * (C - 1)], mybir.dt.float32,
                             kind="ExternalOutput")
    ts_dram = nc.dram_tensor("ts_o", [P, B * (C - 1)], mybir.dt.float32,
                             kind="ExternalOutput")
    psd_dram = nc.dram_tensor("psd_o", [P, C - 1], mybir.dt.float32,
                              kind="ExternalOutput")

    xr = x_dram.ap().rearrange("(b c p) j -> b p c j", b=B, c=C)
    tr = t_dram.ap().rearrange("(b p) j -> b p j", b=B)

    with tile.TileContext(nc) as tc, ExitStack() as ctx:
        xpool = ctx.enter_context(tc.tile_pool(name="x", bufs=4))
        tpool = ctx.enter_context(tc.tile_pool(name="t", bufs=2))
        spool = ctx.enter_context(tc.tile_pool(name="s", bufs=2))
        mpool = ctx.enter_context(tc.tile_pool(name="m", bufs=2))
        apool = ctx.enter_context(tc.tile_pool(name="acc", bufs=1))
        ppool = ctx.enter_context(tc.tile_pool(name="ps", bufs=1,
                                               space="PSUM"))

        ps_cols = apool.tile([P, 2 * B * (C - 1)], mybir.dt.float32)
        ts_cols = apool.tile([P, B * (C - 1)], mybir.dt.float32)
        bias_t = apool.tile([P, C - 1], mybir.dt.float32)
        for i in range(C - 1):
            nc.vector.memset(bias_t[:, i:i + 1], -(i + 0.5))
        psd_cols = apool.tile([P, C - 1], mybir.dt.float32)
        emb_c = apool.tile([P, 5], mybir.dt.int32)
        nc.vector.memset(emb_c[:, 0:1], 7)          # low-bit mask
        for q in range(4):
            nc.vector.memset(emb_c[:, q + 1:q + 2], 2 * q)   # idA per pair
        emb_cf = emb_c[:].bitcast(mybir.dt.float32)
        # one-hot lhsT per batch: ones in column b route the column-sums of
        # each mask chunk into PSUM row b.
        onehot = apool.tile([P, B * B], mybir.dt.bfloat16)
        for b in range(B):
            for j in range(B):
                nc.vector.memset(onehot[:, b * B + j:b * B + j + 1],
                                 1.0 if b == j else 0.0)
        lhsT = [onehot[:, b * B:(b + 1) * B] for b in range(B)]

        # per-class PSUM accumulators [B, MM_N]; class 0 only matters when
        # background is kept (it is sliced away for background=0)
        bins = list(range(0 if with_bin0 else 1, C))
        tp_psum = {c: ppool.tile([B, MM_N], mybir.dt.float32, tag=f"tp_ps{c}",
                                 name=f"tp_ps{c}")
                   for c in bins}

        mx = mybir.AluOpType.max
        sg = mybir.ActivationFunctionType.Sign
        HC = C // 2             # channels per half-load

        SPL = 2 * FD            # psum/mask early-chunk boundary
        QC = 2                  # channels per x load

        def load_x(b, q, f0):
            xt = xpool.tile([P, QC * FD], mybir.dt.float32, tag="xt",
                            name=f"xt_{b}_{q}_{f0}")
            nc.sync.dma_start(xt[:].rearrange("p (c j) -> p c j", c=QC),
                              xr[b, :, q * QC:(q + 1) * QC, f0:f0 + FD])
            return xt

        def load_x1(b, c, f0):
            xt = xpool.tile([P, FD], mybir.dt.float32, tag="xt1",
                            name=f"xt1_{b}_{c}_{f0}", bufs=2)
            nc.sync.dma_start(xt[:].rearrange("p (c j) -> p c j", c=1),
                              xr[b, :, c:c + 1, f0:f0 + FD])
            return xt

        for b in range(B):
            # first x channel before the target DMA so compute starts early
            x00 = load_x1(b, 0, 0)
            x01 = load_x1(b, 1, 0)

            tg = tpool.tile([P, FDC], mybir.dt.int32, tag="tg")
            nc.sync.dma_start(tg[:], tr[b])
            # tsum thresholds as soon as the target lands
            act_dump = spool.tile([P, FDC], mybir.dt.bfloat16, tag="act_dump",
                                  bufs=1)
            for i in range(C - 1):
                nc.scalar.activation(
                    act_dump[:], tg[:], sg, bias=bias_t[:, i:i + 1],
                    scale=1.0,
                    accum_out=ts_cols[:, b * (C - 1) + i:b * (C - 1) + i + 1])

            # comb = 8*tgt + pred, built in bf16 (values 0..63 are exact) so
            # the tp-mask tensor_scalars below hit the DVE 4x perf mode.
            comb = spool.tile([P, FDC], mybir.dt.bfloat16, tag="comb")
            nc.vector.tensor_scalar(comb[:], tg[:], 8, None,
                                    mybir.AluOpType.mult)
            pred = spool.tile([P, FDC], mybir.dt.int32, tag="pred")
            pred_bf = spool.tile([P, FDC], mybir.dt.bfloat16, tag="pred_bf")

            def emit_psum_chunk(lo, hi, col):
                for i in range(C - 1):
                    nc.scalar.activation(
                        act_dump[:, lo:hi], pred[:, lo:hi], sg,
                        bias=bias_t[:, i:i + 1], scale=1.0,
                        accum_out=ps_cols[:, col * (C - 1) + i:
                                          col * (C - 1) + i + 1])

            def emit_mask_chunk(lo, hi, first, last):
                for c in bins:
                    mask = mpool.tile([P, FDC], mybir.dt.bfloat16, tag="mask",
                                      name=f"mask_{b}_{lo}_{c}")
                    nc.vector.tensor_scalar(mask[:, lo:hi], comb[:, lo:hi],
                                            float(9 * c), None,
                                            mybir.AluOpType.is_equal)
                    for k in range(lo // MM_N, hi // MM_N):
                        nc.tensor.matmul(
                            tp_psum[c][:], lhsT[b],
                            mask[:, k * MM_N:(k + 1) * MM_N],
                            start=(first and k == lo // MM_N),
                            stop=(last and k == hi // MM_N - 1))

            for r in range(RPB):
                f0 = r * FD
                m03 = spool.tile([P, FD], mybir.dt.float32, tag="m03")
                for q in range(C // QC):
                    if r == 0 and q == 0:
                        ch = [x00[:], x01[:]]
                    else:
                        xt = load_x(b, q, f0)
                        ch = [xt[:, c * FD:(c + 1) * FD] for c in range(QC)]
                    # fused embed+max folds this channel pair in one DVE op
                    dst = m03[:] if q == 0 else ch[0]
                    nc.vector._custom_dve(
                        _EMBED_MAX, out=dst, in0=ch[0], in1=ch[1],
                        s0=emb_cf[:, 0:1], s1=emb_cf[:, q + 1:q + 2],
                        imm2=_id_bits_float(2 * q + 1))
                    if q > 0:
                        nc.vector.tensor_tensor(m03[:], m03[:], ch[0], mx)

                pr = pred[:, f0:f0 + FD]
                nc.vector.tensor_scalar(pr, m03[:].bitcast(mybir.dt.int32),
                                        7, None, mybir.AluOpType.bitwise_and)
                nc.vector.tensor_copy(pred_bf[:, f0:f0 + FD], pr)
                nc.vector.tensor_tensor(comb[:, f0:f0 + FD],
                                        comb[:, f0:f0 + FD],
                                        pred_bf[:, f0:f0 + FD],
                                        mybir.AluOpType.add)

                if f0 + FD == SPL:
                    emit_psum_chunk(0, SPL, 2 * b)
                    emit_mask_chunk(0, SPL, first=(b == 0), last=False)

            if b < B - 1:
                emit_psum_chunk(SPL, FDC, 2 * b + 1)
            emit_mask_chunk(SPL, FDC, first=False, last=(b == B - 1))
            if b == B - 1:
                # tail chunk on DVE: direct cumulative counts of pred
                pf32 = spool.tile([P, FDC - SPL], mybir.dt.float32,
                                  tag="m03", name="pf32")
                nc.vector.tensor_copy(pf32[:], pred[:, SPL:FDC])
                dved = spool.tile([P, FDC - SPL], mybir.dt.float32,
                                  tag="m03", name="dved")
                for i in range(C - 1):
                    nc.vector.tensor_scalar(
                        dved[:], pf32[:], i + 0.5, None,
                        mybir.AluOpType.is_le, mybir.AluOpType.add,
                        accum_out=psd_cols[:, i:i + 1])

        # drain tp PSUM accumulators: [B, MM_N] -> [B, 1] each
        tp_sb = apool.tile([B, C], mybir.dt.float32)
        nc.vector.memset(tp_sb[:], 0.0)
        for c in bins:
            nc.vector.tensor_reduce(tp_sb[:, c:c + 1], tp_psum[c][:],
                                    mybir.AxisListType.X, mybir.AluOpType.add)
        nc.sync.dma_start(psd_dram.ap(), psd_cols[:])
        nc.sync.dma_start(tp_dram.ap(), tp_sb[:])
        nc.sync.dma_start(ps_dram.ap(), ps_cols[:])
        nc.sync.dma_start(ts_dram.ap(), ts_cols[:])

    nc.compile()
    return nc


def _get_nc(with_bin0=False):
    key = f"nc{int(with_bin0)}"
    if key not in _CACHE:
        _CACHE[key] = _build_nc(with_bin0)
    return _CACHE[key]


def _make_in_maps(input, target):
    x = np.asarray(input, dtype=np.float32).reshape(B, C, N)
    t = np.asarray(target, dtype=np.int32).reshape(B, N)
    in_maps = []
    for k in range(NCORES):
        sl = slice(k * NV, (k + 1) * NV)
        xk = np.ascontiguousarray(x[:, :, sl]).reshape(B * C * P, FDC)
        tk = np.ascontiguousarray(t[:, sl]).reshape(B * P, FDC)
        in_maps.append({"x": xk, "tgt": tk})
    return in_maps


def _postprocess(results, background):
    # Sum partials over cores and partitions (already per-batch columns).
    SPL = 2 * FD
    NA = NCORES * P * SPL          # voxels in the early chunk per batch
    tp = np.zeros((B, C), np.float64)
    ps_ch = np.zeros((B, 2, C - 1), np.float64)
    ts_cols = np.zeros((B, C - 1), np.float64)
    psd = np.zeros(C - 1, np.float64)
    for res in results:
        tp += res["tp_o"].astype(np.float64)
        ps_ch += res["ps_o"].astype(np.float64).sum(0).reshape(B, 2, C - 1)
        ts_cols += res["ts_o"].astype(np.float64).sum(0).reshape(B, C - 1)
        psd += res["psd_o"].astype(np.float64).sum(0)

    psum = np.zeros((B, C), np.float64)
    tsum = np.zeros((B, C), np.float64)
    for b in range(B):
        s = ts_cols[b]                            # S_c = N - 2*count(v <= c)
        f = (N - s) / 2.0
        tsum[b] = np.diff(np.concatenate([[0.0], f, [float(N)]]))
        if b < B - 1:
            s = ps_ch[b].sum(0)
            f = (N - s) / 2.0
        else:
            # early chunk Sign-encoded + tail chunk as direct counts
            f = (NA - ps_ch[b, 0]) / 2.0 + psd
        psum[b] = np.diff(np.concatenate([[0.0], f, [float(N)]]))

    sl = slice(None) if background else slice(1, None)
    tp = tp[:, sl].astype(np.float32)
    psum = psum[:, sl].astype(np.float32)
    tsum = tsum[:, sl].astype(np.float32)
    dice = (np.float32(2.0) * tp / (psum + tsum + np.float32(EPS)))
    return dice.astype(np.float32), tp, psum, tsum


def _run(input, target, background, trace=False, **spmd_kwargs):
    nc = _get_nc(with_bin0=bool(background))
    in_maps = _make_in_maps(input, target)
    res = run_bass_kernel_spmd(nc, in_maps, list(range(NCORES)), trace=trace,
                               **spmd_kwargs)
    return _postprocess(res.results, background), res


def kernel(input, target, background):
    out, _ = _run(input, target, int(np.asarray(background)))
    return out


# revision 32
# speedup vs baseline: 1.1027x; 1.1027x over previous
"""CEDiceMetrics Trainium2 kernel (nn_CEDiceMetrics_69148973466078).

Computes dice/tp/psum/tsum for input [2,8,128,192,192] f32 logits and
target [2,1,128,192,192] int32 labels, sharded over 8 NeuronCores by
splitting the flattened voxel dim.

Per-core device algorithm (tiles of [128 partitions, FD] voxels):
  1. A custom fused DVE op (EMBED_MAX_ANT, declared below and registered
     into the ant custom-DVE table) embeds each channel id into the 3 low
     mantissa bits of the f32 logits and takes the pairwise max in a
     single pass: out = max(a ^ ((a&7) ^ idA), b ^ ((b&7) ^ idB))
     (= (x & ~7) | id on raw bits; XOR form avoids a NaN-pattern
     constant, which HW canonicalizes). All embedded values are distinct
     so the max is tie-free; the <=8-ulp perturbation can only flip
     argmax winners for near-exact ties (~1e-4 relative count error worst
     case, far inside tolerance).
  2. Three more plain f32 max ops fold the four pair-maxes -> m;
     pred = (m as int32) & 7 is the argmax index, exactly.
  3. comb = 8*tgt + pred, kept in bf16 (values 0..63 exact) so the
     per-class tp masks [comb == 9c] run in the DVE 4x perf mode.
  4. tp[c]: bf16 masks are column-summed by the otherwise-idle TensorE
     (ones-vector matmuls accumulating into PSUM; a one-hot lhsT column
     routes each batch to its own PSUM row).
  5. psum/tsum via cumulative counts on the Scalar engine: Sign(v-(c+.5))
     with fused accum_out gives S = N - 2*count(v <= c); class counts are
     differences of adjacent cumulative counts (host side). Chunked so
     the tail threshold work overlaps compute.
Host glue sums the tiny per-core/per-partition partial counts and
evaluates the dice formula. Measured ~156 us on HW vs ~118 us HBM
roofline (42.5 MB/core @ ~358 GB/s); VectorE-bound.
"""

import sys

for _p in ("/root/.axon_site/_ro/trn_rl_repo",):
    if _p not in sys.path:
        sys.path.insert(0, _p)

import numpy as np
from contextlib import ExitStack

import concourse.bacc as bacc
import concourse.mybir as mybir
import concourse.tile as tile
from concourse.bass_utils import run_bass_kernel_spmd
import concourse.dve_ops as _dve_ops
from concourse.dve_ops import DveOp as _DveOp
from concourse.dve_spec import (AluOp as _AluOp, Bin as _Bin, Spec as _Spec,
                                Src0 as _Src0, Src1 as _Src1, C0 as _C0,
                                C1 as _C1, C2 as _C2)


def _embed_max_ref(in0, in1, s0, s1, imm2):
    # z = x ^ ((x & 7) ^ id)  ==  (x & ~7) | id on raw f32 bits
    sev = np.asarray(s0, np.float32).view(np.int32)
    ia = np.asarray(s1, np.float32).view(np.int32)
    ib = np.float32(imm2).view(np.int32)
    xa = np.asarray(in0, np.float32).view(np.int32)
    xb = np.asarray(in1, np.float32).view(np.int32)
    a = (xa ^ ((xa & sev) ^ ia)).view(np.float32)
    b = (xb ^ ((xb & sev) ^ ib)).view(np.float32)
    return np.maximum(a, b)


def _make_embed_max():
    import re as _re
    name = "EMBED_MAX_ANT"
    body = _Bin(
        _AluOp.MAX,
        _Bin(_AluOp.BITWISE_XOR, _Src0,
             _Bin(_AluOp.BITWISE_XOR, _Bin(_AluOp.BITWISE_AND, _Src0, _C0),
                  _C1)),
        _Bin(_AluOp.BITWISE_XOR, _Src1,
             _Bin(_AluOp.BITWISE_XOR, _Bin(_AluOp.BITWISE_AND, _Src1, _C0),
                  _C2)),
    )
    spec = _Spec(body=body, reference=_embed_max_ref)
    for op in _dve_ops.OPS:
        if op.name == name:
            return op
    if name not in _dve_ops._SUB_OPCODE_FOR_NAME:
        _dve_ops._SUB_OPCODE_FOR_NAME[name] = (
            max(_dve_ops._SUB_OPCODE_FOR_NAME.values()) + 1)
    probe = _DveOp(name, spec, subdim=False, uops_sha={})
    shas = {}
    try:
        probe.compile("v3")
    except ValueError as e:
        shas["v3"] = _re.search(r"v3: (\w+)", str(e)).group(1)
    op = _DveOp(name, spec, subdim=False, uops_sha=shas)
    _dve_ops.OPS.append(op)
    _dve_ops.CUSTOM_DVE_SPECS[name] = spec
    return op


_EMBED_MAX = _make_embed_max()


def _id_bits_float(i):
    return float(np.int32(i).view(np.float32))

# Problem geometry (hardcoded per spec).
B, C = 2, 8
D, H, W = 128, 192, 192
N = D * H * W                 # 4,718,592 voxels per (b, c)
NCORES = 8
NV = N // NCORES              # 589,824 voxels per core per batch
P = 128
FDC = NV // P                 # 4,608 free elems per partition per batch
FD = 1536                     # free elems per round
RPB = FDC // FD               # 3 rounds per batch
R = B * RPB                   # 6 rounds per core
EPS = 1e-5

_CACHE = {}


MM_N = 512                      # PSUM bank width in f32; matmul chunk
MM_CHUNKS = FDC // MM_N         # matmul chunks per batch-level mask


def _build_nc(with_bin0=False):
    nc = bacc.Bacc("TRN2", target_bir_lowering=False, debug=False,
                   num_devices=NCORES)
    x_dram = nc.dram_tensor("x", [B * C * P, FDC], mybir.dt.float32,
                            kind="ExternalInput")
    t_dram = nc.dram_tensor("tgt", [B * P, FDC], mybir.dt.int32,
                            kind="ExternalInput")
    tp_dram = nc.dram_tensor("tp_o", [B, C], mybir.dt.float32,
                             kind="ExternalOutput")
    ps_dram = nc.dram_tensor("ps_o", [P, 2 * B * (C - 1)], mybir.dt.float32,
                             kind="ExternalOutput")
    ts_dram = nc.dram_tensor("ts_o", [P, B * (C - 1)], mybir.dt.float32,
                             kind="ExternalOutput")

    xr = x_dram.ap().rearrange("(b c p) j -> b p c j", b=B, c=C)
    tr = t_dram.ap().rearrange("(b p) j -> b p j", b=B)

    with tile.TileContext(nc) as tc, ExitStack() as ctx:
        xpool = ctx.enter_context(tc.tile_pool(name="x", bufs=4))
        tpool = ctx.enter_context(tc.tile_pool(name="t", bufs=2))
        spool = ctx.enter_context(tc.tile_pool(name="s", bufs=2))
        mpool = ctx.enter_context(tc.tile_pool(name="m", bufs=2))
        apool = ctx.enter_context(tc.tile_pool(name="acc", bufs=1))
        ppool = ctx.enter_context(tc.tile_pool(name="ps", bufs=1,
                                               space="PSUM"))

        ps_cols = apool.tile([P, 2 * B * (C - 1)], mybir.dt.float32)
        ts_cols = apool.tile([P, B * (C - 1)], mybir.dt.float32)
        bias_t = apool.tile([P, C - 1], mybir.dt.float32)
        for i in range(C - 1):
            nc.vector.memset(bias_t[:, i:i + 1], -(i + 0.5))
        emb_c = apool.tile([P, 5], mybir.dt.int32)
        nc.vector.memset(emb_c[:, 0:1], 7)          # low-bit mask
        for q in range(4):
            nc.vector.memset(emb_c[:, q + 1:q + 2], 2 * q)   # idA per pair
        emb_cf = emb_c[:].bitcast(mybir.dt.float32)
        # one-hot lhsT per batch: ones in column b route the column-sums of
        # each mask chunk into PSUM row b.
        onehot = apool.tile([P, B * B], mybir.dt.bfloat16)
        for b in range(B):
            for j in range(B):
                nc.vector.memset(onehot[:, b * B + j:b * B + j + 1],
                                 1.0 if b == j else 0.0)
        lhsT = [onehot[:, b * B:(b + 1) * B] for b in range(B)]

        # per-class PSUM accumulators [B, MM_N]; class 0 only matters when
        # background is kept (it is sliced away for background=0)
        bins = list(range(0 if with_bin0 else 1, C))
        tp_psum = {c: ppool.tile([B, MM_N], mybir.dt.float32, tag=f"tp_ps{c}",
                                 name=f"tp_ps{c}")
                   for c in bins}

        mx = mybir.AluOpType.max
        sg = mybir.ActivationFunctionType.Sign
        HC = C // 2             # channels per half-load

        SPL = 2 * FD            # psum/mask early-chunk boundary
        QC = 2                  # channels per x load

        def load_x(b, q, f0):
            xt = xpool.tile([P, QC * FD], mybir.dt.float32, tag="xt",
                            name=f"xt_{b}_{q}_{f0}")
            nc.sync.dma_start(xt[:].rearrange("p (c j) -> p c j", c=QC),
                              xr[b, :, q * QC:(q + 1) * QC, f0:f0 + FD])
            return xt

        def load_x1(b, c, f0):
            xt = xpool.tile([P, FD], mybir.dt.float32, tag="xt1",
                            name=f"xt1_{b}_{c}_{f0}", bufs=2)
            nc.sync.dma_start(xt[:].rearrange("p (c j) -> p c j", c=1),
                              xr[b, :, c:c + 1, f0:f0 + FD])
            return xt

        for b in range(B):
            # first x channel before the target DMA so compute starts early
            x00 = load_x1(b, 0, 0)
            x01 = load_x1(b, 1, 0)

            tg = tpool.tile([P, FDC], mybir.dt.int32, tag="tg")
            nc.sync.dma_start(tg[:], tr[b])
            # tsum thresholds as soon as the target lands
            act_dump = spool.tile([P, FDC], mybir.dt.bfloat16, tag="act_dump",
                                  bufs=1)
            for i in range(C - 1):
                nc.scalar.activation(
                    act_dump[:], tg[:], sg, bias=bias_t[:, i:i + 1],
                    scale=1.0,
                    accum_out=ts_cols[:, b * (C - 1) + i:b * (C - 1) + i + 1])

            # comb = 8*tgt + pred, built in bf16 (values 0..63 are exact) so
            # the tp-mask tensor_scalars below hit the DVE 4x perf mode.
            comb = spool.tile([P, FDC], mybir.dt.bfloat16, tag="comb")
            nc.vector.tensor_scalar(comb[:], tg[:], 8, None,
                                    mybir.AluOpType.mult)
            pred = spool.tile([P, FDC], mybir.dt.int32, tag="pred")
            pred_bf = spool.tile([P, FDC], mybir.dt.bfloat16, tag="pred_bf")

            def emit_psum_chunk(lo, hi, col):
                for i in range(C - 1):
                    nc.scalar.activation(
                        act_dump[:, lo:hi], pred[:, lo:hi], sg,
                        bias=bias_t[:, i:i + 1], scale=1.0,
                        accum_out=ps_cols[:, col * (C - 1) + i:
                                          col * (C - 1) + i + 1])

            def emit_mask_chunk(lo, hi, first, last):
                for c in bins:
                    mask = mpool.tile([P, FDC], mybir.dt.bfloat16, tag="mask",
                                      name=f"mask_{b}_{lo}_{c}")
                    nc.vector.tensor_scalar(mask[:, lo:hi], comb[:, lo:hi],
                                            float(9 * c), None,
                                            mybir.AluOpType.is_equal)
                    for k in range(lo // MM_N, hi // MM_N):
                        nc.tensor.matmul(
                            tp_psum[c][:], lhsT[b],
                            mask[:, k * MM_N:(k + 1) * MM_N],
                            start=(first and k == lo // MM_N),
                            stop=(last and k == hi // MM_N - 1))

            for r in range(RPB):
                f0 = r * FD
                m03 = spool.tile([P, FD], mybir.dt.float32, tag="m03")
                for q in range(C // QC):
                    if r == 0 and q == 0:
                        ch = [x00[:], x01[:]]
                    else:
                        xt = load_x(b, q, f0)
                        ch = [xt[:, c * FD:(c + 1) * FD] for c in range(QC)]
                    # fused embed+max folds this channel pair in one DVE op
                    dst = m03[:] if q == 0 else ch[0]
                    nc.vector._custom_dve(
                        _EMBED_MAX, out=dst, in0=ch[0], in1=ch[1],
                        s0=emb_cf[:, 0:1], s1=emb_cf[:, q + 1:q + 2],
                        imm2=_id_bits_float(2 * q + 1))
                    if q > 0:
                        nc.vector.tensor_tensor(m03[:], m03[:], ch[0], mx)

                pr = pred[:, f0:f0 + FD]
                nc.vector.tensor_scalar(pr, m03[:].bitcast(mybir.dt.int32),
                                        7, None, mybir.AluOpType.bitwise_and)
                nc.vector.tensor_copy(pred_bf[:, f0:f0 + FD], pr)
                nc.vector.tensor_tensor(comb[:, f0:f0 + FD],
                                        comb[:, f0:f0 + FD],
                                        pred_bf[:, f0:f0 + FD],
                                        mybir.AluOpType.add)

                if f0 + FD == SPL:
                    emit_psum_chunk(0, SPL, 2 * b)
                    emit_mask_chunk(0, SPL, first=(b == 0), last=False)

            emit_psum_chunk(SPL, FDC, 2 * b + 1)
            emit_mask_chunk(SPL, FDC, first=False, last=(b == B - 1))

        # drain tp PSUM accumulators: [B, MM_N] -> [B, 1] each
        tp_sb = apool.tile([B, C], mybir.dt.float32)
        nc.vector.memset(tp_sb[:], 0.0)
        for c in bins:
            nc.vector.tensor_reduce(tp_sb[:, c:c + 1], tp_psum[c][:],
                                    mybir.AxisListType.X, mybir.AluOpType.add)
        nc.sync.dma_start(tp_dram.ap(), tp_sb[:])
        nc.sync.dma_start(ps_dram.ap(), ps_cols[:])
        nc.sync.dma_start(ts_dram.ap(), ts_cols[:])

    nc.compile()
    return nc


def _get_nc(with_bin0=False):
    key = f"nc{int(with_bin0)}"
    if key not in _CACHE:
        _CACHE[key] = _build_nc(with_bin0)
    return _CACHE[key]


def _make_in_maps(input, target):
    x = np.asarray(input, dtype=np.float32).reshape(B, C, N)
    t = np.asarray(target, dtype=np.int32).reshape(B, N)
    in_maps = []
    for k in range(NCORES):
        sl = slice(k * NV, (k + 1) * NV)
        xk = np.ascontiguousarray(x[:, :, sl]).reshape(B * C * P, FDC)
        tk = np.ascontiguousarray(t[:, sl]).reshape(B * P, FDC)
        in_maps.append({"x": xk, "tgt": tk})
    return in_maps


def _postprocess(results, background):
    # Sum partials over cores and partitions (already per-batch columns).
    tp = np.zeros((B, C), np.float64)
    ps_cols = np.zeros((B, C - 1), np.float64)
    ts_cols = np.zeros((B, C - 1), np.float64)
    for res in results:
        tp += res["tp_o"].astype(np.float64)
        ps_cols += (res["ps_o"].astype(np.float64).sum(0)
                    .reshape(B, 2, C - 1).sum(1))
        ts_cols += res["ts_o"].astype(np.float64).sum(0).reshape(B, C - 1)

    psum = np.zeros((B, C), np.float64)
    tsum = np.zeros((B, C), np.float64)
    for b in range(B):
        for cum, out in ((ps_cols, psum), (ts_cols, tsum)):
            s = cum[b]                            # S_c = N - 2*count(v <= c)
            f = (N - s) / 2.0                     # count(v <= c), c = 0..6
            full = np.concatenate([[0.0], f, [float(N)]])
            out[b] = np.diff(full)

    sl = slice(None) if background else slice(1, None)
    tp = tp[:, sl].astype(np.float32)
    psum = psum[:, sl].astype(np.float32)
    tsum = tsum[:, sl].astype(np.float32)
    dice = (np.float32(2.0) * tp / (psum + tsum + np.float32(EPS)))
    return dice.astype(np.float32), tp, psum, tsum


def _run(input, target, background, trace=False, **spmd_kwargs):
    nc = _get_nc(with_bin0=bool(background))
    in_maps = _make_in_maps(input, target)
    res = run_bass_kernel_spmd(nc, in_maps, list(range(NCORES)), trace=trace,
                               **spmd_kwargs)
    return _postprocess(res.results, background), res


def kernel(input, target, background):
    out, _ = _run(input, target, int(np.asarray(background)))
    return out
